# revision 1
# baseline (speedup 1.0000x reference)
"""Trainium2 Bass kernel for nn_DMFMLayer (Mamba-style selective-scan block).

Sharding: 2 branches x 4 batch = 8 independent scan units -> one per core.
Each core runs the full mamba chain for its (branch, batch) pair in
(feature-major) layout: d on partitions, L on the free dim, so the
sequential scan maps onto the DVE's tensor_tensor_scan instruction.
"""
import sys, json

sys.path.insert(0, '/opt/trn_rl_repo')
import numpy as np
import concourse.bass as bass
import concourse.mybir as mybir
from concourse.tile import TileContext
from concourse.bass_utils import run_bass_kernel_spmd

F32 = mybir.dt.float32
BF16 = mybir.dt.bfloat16
AF = mybir.ActivationFunctionType
OP = mybir.AluOpType

B, C, W_, H_ = 4, 128, 64, 64
L = W_ * H_              # 4096
DI = 2 * C               # 256 (d_inner), 2 partition blocks
DT_RANK = 8
N_STATE = 16
D_CONV = 4
GROUP = 8
LC = 512                 # L chunk
NCHUNK = L // LC
EPS = 1e-5


def _split_waits(js: bytes, max_waits: int = 1) -> bytes:
    """This walrus build allows only one sync-wait per instruction; move
    excess waits onto EventSemaphore instructions inserted just before."""
    obj = json.loads(js)

    def fix_list(lst):
        out = []
        for item in lst:
            if isinstance(item, dict) and "opcode" in item and isinstance(item.get("sync_info"), dict):
                waits = item["sync_info"].get("on_wait") or []
                if len(waits) > max_waits:
                    excess, keep = waits[:-max_waits], waits[-max_waits:]
                    for k, w in enumerate(excess):
                        out.append({
                            "engine": item.get("engine"), "ins": [], "outs": [],
                            "name": f"{item.get('name', 'I')}_sw{k}",
                            "opcode": "EventSemaphore",
                            "sync_info": {"on_update": [], "on_wait": [w]},
                        })
                    item["sync_info"]["on_wait"] = keep
            out.append(item)
        return out

    def walk(o):
        if isinstance(o, dict):
            for k, v in o.items():
                if isinstance(v, list) and any(isinstance(x, dict) and "opcode" in x for x in v):
                    o[k] = fix_list(v)
                else:
                    walk(v)
        elif isinstance(o, list):
            for v in o:
                walk(v)

    walk(obj)
    return json.dumps(obj).encode()


def _bcast_row(ap):
    """AP reading one SBUF row replicated across 128 partitions (DMA only)."""
    return bass.AP(tensor=ap.tensor, offset=ap.offset, ap=[[0, 128]] + ap.ap[1:])


def build_nc(a_vals, bf16=True):
    """a_vals: 16 floats, A[n] = -exp(A_log[0, n]) baked as exp() scales."""
    SDT = BF16 if bf16 else F32
    nc = bass.Bass()
    xin = nc.dram_tensor("xin", [C, L], F32, kind="ExternalInput")
    w_in_T = nc.dram_tensor("w_in_T", [C, 2 * DI], F32, kind="ExternalInput")
    wx_T = nc.dram_tensor("wx_T", [2, 128, DT_RANK + 2 * N_STATE], F32, kind="ExternalInput")
    wdt_T = nc.dram_tensor("wdt_T", [DT_RANK, DI], F32, kind="ExternalInput")
    wout_T = nc.dram_tensor("wout_T", [2, 128, C], F32, kind="ExternalInput")
    convw = nc.dram_tensor("convw", [2, 128, D_CONV], F32, kind="ExternalInput")
    convb = nc.dram_tensor("convb", [2, 128, 1], F32, kind="ExternalInput")
    nconvb = nc.dram_tensor("nconvb", [2, 128, 1], F32, kind="ExternalInput")
    bdt = nc.dram_tensor("bdt", [2, 128, 1], F32, kind="ExternalInput")
    dskip = nc.dram_tensor("dskip", [2, 128, 1], F32, kind="ExternalInput")
    svec = nc.dram_tensor("svec", [128, 1], F32, kind="ExternalInput")
    sel = None if bf16 else nc.dram_tensor("sel", [2 * N_STATE, 2 * N_STATE * 128], F32, kind="ExternalInput")
    ident = nc.dram_tensor("ident", [128, 128], F32, kind="ExternalInput")
    identd = nc.dram_tensor("identd", [2, 128, 128], F32, kind="ExternalInput")
    idents = nc.dram_tensor("idents", [128, 128], F32, kind="ExternalInput")
    xbc_dram = nc.dram_tensor("xbc_scratch", [NCHUNK, 2 * N_STATE, LC], mybir.dt.bfloat16, kind="Internal") if bf16 else None
    pout = nc.dram_tensor("pout", [C, L], F32, kind="ExternalOutput")

    with TileContext(nc) as tc:
        with (
            tc.tile_pool(name="singles", bufs=1) as singles,
            tc.tile_pool(name="work", bufs=2) as work,
            tc.tile_pool(name="psum", bufs=1, space="PSUM") as psum,
        ):
            # ---- persistent weights -------------------------------------
            w_in_sb = singles.tile([C, 2 * DI], F32, tag="w_in", name="w_in")
            nc.sync.dma_start(out=w_in_sb, in_=w_in_T[:, :])
            wx_sb = [singles.tile([128, DT_RANK + 2 * N_STATE], F32, tag=f"wx{i}", name=f"wx{i}") for i in range(2)]
            wdt_sb = singles.tile([DT_RANK, DI], F32, tag="wdt", name="wdt")
            nc.sync.dma_start(out=wdt_sb, in_=wdt_T[:, :])
            wout_sb = [singles.tile([128, C], F32, tag=f"wo{i}", name=f"wo{i}") for i in range(2)]
            convw_sb = [singles.tile([128, D_CONV], F32, tag=f"cw{i}", name=f"cw{i}") for i in range(2)]
            convb_sb = [singles.tile([128, 1], F32, tag=f"cb{i}", name=f"cb{i}") for i in range(2)]
            nconvb_sb = [singles.tile([128, 1], F32, tag=f"ncb{i}", name=f"ncb{i}") for i in range(2)]
            bdt_sb = [singles.tile([128, 1], F32, tag=f"bd{i}", name=f"bd{i}") for i in range(2)]
            dskip_sb = [singles.tile([128, 1], F32, tag=f"ds{i}", name=f"ds{i}") for i in range(2)]
            for i in range(2):
                nc.sync.dma_start(out=wx_sb[i], in_=wx_T[i, :, :])
                nc.sync.dma_start(out=wout_sb[i], in_=wout_T[i, :, :])
                nc.sync.dma_start(out=convw_sb[i], in_=convw[i, :, :])
                nc.sync.dma_start(out=convb_sb[i], in_=convb[i, :, :])
                nc.sync.dma_start(out=nconvb_sb[i], in_=nconvb[i, :, :])
                nc.sync.dma_start(out=bdt_sb[i], in_=bdt[i, :, :])
                nc.sync.dma_start(out=dskip_sb[i], in_=dskip[i, :, :])
            svec_sb = singles.tile([128, 1], F32, tag="sv", name="sv")
            nc.sync.dma_start(out=svec_sb, in_=svec[:, :])
            if not bf16:
                sel_sb = singles.tile([2 * N_STATE, 2 * N_STATE * 128], F32, tag="sel", name="sel")
                nc.sync.dma_start(out=sel_sb, in_=sel[:, :])
            identd_sb = [singles.tile([128, 128], F32, tag=f"idd{i}", name=f"idd{i}") for i in range(2)]
            for i in range(2):
                nc.sync.dma_start(out=identd_sb[i], in_=identd[i, :, :])
            idents_sb = singles.tile([128, 128], F32, tag="ids", name="ids")
            nc.sync.dma_start(out=idents_sb, in_=idents[:, :])
            id_sb = singles.tile([128, 128], SDT, tag="ident", name="ident")
            if bf16:
                idf = singles.tile([128, 128], F32, tag="identf", name="identf")
                nc.sync.dma_start(out=idf, in_=ident[:, :])
                nc.vector.tensor_copy(id_sb, idf)
            else:
                nc.sync.dma_start(out=id_sb, in_=ident[:, :])
            hstate = [singles.tile([128, N_STATE], SDT, tag=f"hs{i}", name=f"hs{i}") for i in range(2)]
            ones_lc = singles.tile([128, LC], F32, tag="ones_lc", name="ones_lc")
            nc.vector.memset(ones_lc, 1.0)
            cwrep = []
            for i in range(2):
                row = []
                for k in range(D_CONV):
                    t = singles.tile([128, LC], F32, tag=f"cwr{i}{k}", name=f"cwr{i}{k}")
                    nc.vector.tensor_scalar(t, ones_lc, convw_sb[i][:, k:k + 1], None, op0=OP.mult)
                    row.append(t)
                cwrep.append(row)
            hist = [singles.tile([128, D_CONV - 1], F32, tag=f"hi{i}", name=f"hi{i}") for i in range(2)]
            for i in range(2):
                nc.vector.memset(hist[i], 0.0)

            # ---- main loop over L chunks --------------------------------
            for c in range(NCHUNK):
                sl = slice(c * LC, (c + 1) * LC)
                x_c = work.tile([C, LC], F32, tag="xc", name="xc")
                nc.sync.dma_start(out=x_c, in_=xin[:, sl])

                # in_proj: xz = W_in' @ x  (4 output blocks of 128)
                xi_pad = [work.tile([128, LC + D_CONV - 1], F32, tag=f"xip{i}", name=f"xip{i}") for i in range(2)]
                sz = [work.tile([128, LC], F32, tag=f"sz{i}", name=f"sz{i}") for i in range(2)]
                for j in range(4):
                    pxz = psum.tile([128, LC], F32, tag="xz", name="xz", bufs=2)
                    nc.tensor.matmul(pxz, w_in_sb[:, j * 128:(j + 1) * 128], x_c, start=True, stop=True)
                    if j < 2:
                        nc.scalar.copy(xi_pad[j][:, D_CONV - 1:], pxz)
                    else:
                        nc.scalar.activation(sz[j - 2], pxz, AF.Silu)

                # causal depthwise conv + silu -> u
                u = [work.tile([128, LC], F32, tag=f"u{i}", name=f"u{i}") for i in range(2)]
                preu = [work.tile([128, LC], F32, tag=f"pu{i}", name=f"pu{i}") for i in range(2)]
                cacc = [work.tile([128, LC], F32, tag=f"ca{i}", name=f"ca{i}", bufs=1) for i in range(2)]
                for i in range(2):
                    nc.gpsimd.tensor_copy(xi_pad[i][:, 0:D_CONV - 1], hist[i])
                    nc.gpsimd.tensor_tensor(out=preu[i], in0=xi_pad[i][:, 0:LC], in1=cwrep[i][0], op=OP.mult)
                    for k in range(1, D_CONV):
                        nc.gpsimd.tensor_tensor(out=cacc[i], in0=xi_pad[i][:, k:k + LC], in1=cwrep[i][k], op=OP.mult)
                        nc.gpsimd.tensor_tensor(out=preu[i], in0=preu[i], in1=cacc[i], op=OP.add)
                    nc.gpsimd.tensor_copy(hist[i], xi_pad[i][:, LC:LC + D_CONV - 1])
                    nc.scalar.activation(u[i], preu[i], AF.Silu, bias=convb_sb[i][:, 0:1])

                # x_dbl = W_x @ u  -> dt rows (8, LC) and B|C rows (32, LC)
                pxd = psum.tile([DT_RANK, LC], F32, tag="mm", name="xd", bufs=2)
                nc.tensor.matmul(pxd, wx_sb[0][:, 0:DT_RANK], u[0], start=True, stop=False)
                nc.tensor.matmul(pxd, wx_sb[1][:, 0:DT_RANK], u[1], start=False, stop=True)
                xdbl = work.tile([DT_RANK, LC], F32, tag="xdbl", name="xdbl")
                nc.scalar.copy(xdbl, pxd)
                pbc = psum.tile([2 * N_STATE, LC], F32, tag="mm", name="bc", bufs=2)
                nc.tensor.matmul(pbc, wx_sb[0][:, DT_RANK:], u[0], start=True, stop=False)
                nc.tensor.matmul(pbc, wx_sb[1][:, DT_RANK:], u[1], start=False, stop=True)
                if bf16:
                    xbc = work.tile([2 * N_STATE, LC], BF16, tag="xbc", name="xbc")
                    nc.scalar.copy(xbc, pbc)
                    nc.sync.dma_start(out=xbc_dram[c, :, :], in_=xbc)
                    brep_all = work.tile([128, N_STATE * LC], BF16, tag="brepall", name="brepall")
                    crep_all = work.tile([128, N_STATE * LC], BF16, tag="crepall", name="crepall")
                    base = xbc_dram[c, 0, 0]
                    nc.sync.dma_start(out=brep_all, in_=bass.AP(
                        tensor=base.tensor, offset=base.offset, ap=[[0, 128], [LC, N_STATE], [1, LC]]))
                    nc.sync.dma_start(out=crep_all, in_=bass.AP(
                        tensor=base.tensor, offset=base.offset + N_STATE * LC, ap=[[0, 128], [LC, N_STATE], [1, LC]]))
                else:
                    xbc = work.tile([2 * N_STATE, LC], F32, tag="xbc", name="xbc")
                    nc.vector.tensor_copy(xbc, pbc)

                # dt = softplus(dt_in @ W_dt.T + b_dt); dtu = dt * u
                dt_c = [work.tile([128, LC], F32, tag=f"dt{i}", name=f"dt{i}") for i in range(2)]
                dtu = [work.tile([128, LC], SDT, tag=f"du{i}", name=f"du{i}") for i in range(2)]
                for i in range(2):
                    pdt = psum.tile([128, LC], F32, tag="mm", name="dtp", bufs=2)
                    nc.tensor.matmul(pdt, wdt_sb[:, i * 128:(i + 1) * 128], xdbl, start=True, stop=True)
                    edt = work.tile([128, LC], F32, tag=f"edt{i}", name=f"edt{i}", bufs=1)
                    nc.scalar.activation(edt, pdt, AF.Exp, bias=bdt_sb[i][:, 0:1])
                    nc.scalar.activation(dt_c[i], edt, AF.Ln, bias=1.0)
                    nc.gpsimd.tensor_tensor(out=dtu[i], in0=dt_c[i], in1=u[i], op=OP.mult)

                # selective scan over the chunk, one (n, dblk) recurrence per row
                py = [psum.tile([128, LC], F32, tag=f"py{i}", name=f"py{i}") for i in range(2)]
                for n in range(N_STATE):
                    if bf16:
                        pb = brep_all[:, n * LC:(n + 1) * LC]
                        pc = crep_all[:, n * LC:(n + 1) * LC]
                    else:
                        pb = psum.tile([128, LC], F32, tag="brep", name="brep", bufs=2)
                        nc.tensor.matmul(pb, sel_sb[:, n * 128:(n + 1) * 128], xbc, start=True, stop=True)
                        pc = psum.tile([128, LC], F32, tag="crep", name="crep", bufs=2)
                        nc.tensor.matmul(pc, sel_sb[:, (N_STATE + n) * 128:(N_STATE + n + 1) * 128], xbc, start=True, stop=True)
                    for i in range(2):
                        dA = work.tile([128, LC], SDT, tag=f"dA{i}", name=f"dA{i}")
                        nc.scalar.activation(dA, dt_c[i], AF.Exp, scale=float(a_vals[n]))
                        bt = work.tile([128, LC], SDT, tag=f"bt{i}", name=f"bt{i}")
                        nc.vector.tensor_tensor(out=bt, in0=dtu[i], in1=pb, op=OP.mult)
                        hsl = work.tile([128, LC], SDT, tag=f"h{i}", name=f"h{i}")
                        init = 0.0 if c == 0 else hstate[i][:, n:n + 1]
                        nc.vector.tensor_tensor_scan(out=hsl, data0=dA, data1=bt, initial=init, op0=OP.mult, op1=OP.add)
                        if c < NCHUNK - 1:
                            nc.vector.tensor_copy(hstate[i][:, n:n + 1], hsl[:, LC - 1:LC])
                        p = work.tile([128, LC], SDT, tag=f"p{i}", name=f"p{i}")
                        peng = nc.gpsimd if bf16 and ((n * 2 + i) % 2 == 1) else nc.vector
                        peng.tensor_tensor(out=p, in0=hsl, in1=pc, op=OP.mult)
                        nc.tensor.matmul(py[i], id_sb, p, start=(n == 0), stop=False, skip_group_check=True)
                # y += u * D_skip (via PE accum); g = y * silu(z); out = W_out @ g + s * x
                po = psum.tile([C, LC], F32, tag="op", name="op")
                for i in range(2):
                    nc.tensor.matmul(py[i], identd_sb[i], u[i], start=False, stop=True, skip_group_check=True)
                    g = work.tile([128, LC], F32, tag=f"g{i}", name=f"g{i}", bufs=1)
                    nc.vector.tensor_tensor(out=g, in0=sz[i], in1=py[i], op=OP.mult)
                    nc.tensor.matmul(po, wout_sb[i], g, start=(i == 0), stop=False, skip_group_check=True)
                nc.tensor.matmul(po, idents_sb, x_c, start=False, stop=True, skip_group_check=True)
                out_sb = work.tile([C, LC], F32, tag="osb", name="osb")
                nc.scalar.copy(out_sb, po)
                nc.sync.dma_start(out=pout[:, sl], in_=out_sb)

    orig = nc.to_json_bytes
    nc.to_json_bytes = lambda: _split_waits(orig())
    return nc




LH = L // 2  # 2048, half-sequence per finale core


def build_nc2():
    """Phase 2: xm = p0+p1; LN over C (partition dim, via PE stats); out = W_p@(...)+b."""
    nc = bass.Bass()
    pa = nc.dram_tensor("pa", [C, LH], F32, kind="ExternalInput")
    pb = nc.dram_tensor("pb", [C, LH], F32, kind="ExternalInput")
    wpg_T = nc.dram_tensor("wpg_T", [C, C], F32, kind="ExternalInput")
    w1r = nc.dram_tensor("w1r", [1, C], F32, kind="ExternalInput")
    wbp = nc.dram_tensor("wbp", [1, C], F32, kind="ExternalInput")
    fout = nc.dram_tensor("fout", [C, LH], F32, kind="ExternalOutput")

    with TileContext(nc) as tc:
        with (
            tc.tile_pool(name="sgl", bufs=1) as sgl,
            tc.tile_pool(name="wrk", bufs=1) as wrk,
            tc.tile_pool(name="ps", bufs=1, space="PSUM") as ps,
        ):
            wpg_sb = sgl.tile([C, C], F32, tag="wpg", name="wpg")
            nc.sync.dma_start(out=wpg_sb, in_=wpg_T[:, :])
            w1_sb = sgl.tile([1, C], F32, tag="w1", name="w1")
            nc.sync.dma_start(out=w1_sb, in_=w1r[:, :])
            wbp_sb = sgl.tile([1, C], F32, tag="wbp", name="wbp")
            nc.sync.dma_start(out=wbp_sb, in_=wbp[:, :])
            ones_col = sgl.tile([C, 1], F32, tag="onesc", name="onesc")
            nc.vector.memset(ones_col, 1.0)
            ones_row = sgl.tile([1, LH], F32, tag="onesr", name="onesr")
            nc.vector.memset(ones_row, 1.0)
            ones_r128 = sgl.tile([1, 128], F32, tag="onesr128", name="onesr128")
            nc.vector.memset(ones_r128, 1.0)
            eps_col = sgl.tile([128, 1], F32, tag="epsc", name="epsc")
            nc.vector.memset(eps_col, EPS)

            xm = wrk.tile([C, LH], F32, tag="xm", name="xm")

            # per-512-chunk LN stats + apply, fully pipelined
            out_sb = wrk.tile([C, LH], F32, tag="osb2", name="osb2")
            NTq = 512 // 128
            for q in range(LH // 512):
                qs = slice(q * 512, (q + 1) * 512)
                pa_sb = wrk.tile([C, 512], F32, tag="pa", name="pa", bufs=2)
                nc.sync.dma_start(out=pa_sb, in_=pa[:, qs])
                pb_sb = wrk.tile([C, 512], F32, tag="pb", name="pb", bufs=2)
                nc.sync.dma_start(out=pb_sb, in_=pb[:, qs])
                nc.vector.tensor_tensor(out=xm[:, qs], in0=pa_sb, in1=pb_sb, op=OP.add)
                xsq = wrk.tile([C, 512], F32, tag="xsq", name="xsq", bufs=2)
                nc.scalar.activation(xsq, xm[:, qs], AF.Square)
                s1p = ps.tile([1, 512], F32, tag="s1", name="s1", bufs=2)
                nc.tensor.matmul(s1p, ones_col, xm[:, qs], start=True, stop=True)
                s1q = wrk.tile([1, 512], F32, tag="s1q", name="s1q", bufs=2)
                nc.vector.tensor_copy(s1q, s1p)
                s2p = ps.tile([1, 512], F32, tag="s2", name="s2", bufs=2)
                nc.tensor.matmul(s2p, ones_col, xsq, start=True, stop=True)
                s2q = wrk.tile([1, 512], F32, tag="s2q", name="s2q", bufs=2)
                nc.vector.tensor_copy(s2q, s2p)
                s1t = wrk.tile([128, NTq], F32, tag="s1t", name="s1t", bufs=2)
                nc.sync.dma_start(out=s1t, in_=s1q)
                s2t = wrk.tile([128, NTq], F32, tag="s2t", name="s2t", bufs=2)
                nc.sync.dma_start(out=s2t, in_=s2q)
                mu = wrk.tile([128, NTq], F32, tag="mu", name="mu", bufs=2)
                nc.vector.tensor_scalar(mu, s1t, 1.0 / C, None, op0=OP.mult)
                m2 = wrk.tile([128, NTq], F32, tag="m2", name="m2", bufs=2)
                nc.vector.tensor_scalar(m2, s2t, 1.0 / C, None, op0=OP.mult)
                musq = wrk.tile([128, NTq], F32, tag="musq", name="musq", bufs=2)
                nc.vector.tensor_tensor(out=musq, in0=mu, in1=mu, op=OP.mult)
                var = wrk.tile([128, NTq], F32, tag="var", name="var", bufs=2)
                nc.vector.tensor_tensor(out=m2, in0=m2, in1=musq, op=OP.subtract)
                nc.scalar.activation(var, m2, AF.Sqrt, bias=eps_col[:, 0:1])
                rs = wrk.tile([128, NTq], F32, tag="rs", name="rs", bufs=2)
                nc.vector.reciprocal(rs, var)
                ms = wrk.tile([128, NTq], F32, tag="ms", name="ms", bufs=2)
                nc.vector.tensor_tensor(out=ms, in0=mu, in1=rs, op=OP.mult)
                nc.vector.tensor_scalar(ms, ms, -1.0, None, op0=OP.mult)
                s_row = wrk.tile([1, 512], F32, tag="s_row", name="s_row", bufs=2)
                nc.sync.dma_start(out=s_row, in_=rs)
                ms_row = wrk.tile([1, 512], F32, tag="ms_row", name="ms_row", bufs=2)
                nc.sync.dma_start(out=ms_row, in_=ms)
                srep = ps.tile([C, 512], F32, tag="srep", name="srep", bufs=2)
                nc.tensor.matmul(srep, ones_r128, s_row, start=True, stop=True)
                xms = wrk.tile([C, 512], F32, tag="xms", name="xms", bufs=2)
                nc.vector.tensor_tensor(out=xms, in0=xm[:, qs], in1=srep, op=OP.mult)
                pout2 = ps.tile([C, 512], F32, tag="po2", name="po2", bufs=2)
                nc.tensor.matmul(pout2, wpg_sb, xms, start=True, stop=False)
                nc.tensor.matmul(pout2, w1_sb, ms_row, start=False, stop=False, skip_group_check=True)
                nc.tensor.matmul(pout2, wbp_sb, ones_row[:, qs], start=False, stop=True, skip_group_check=True)
                nc.scalar.copy(out_sb[:, qs], pout2)
            nc.sync.dma_start(out=fout[:, :], in_=out_sb)

    orig = nc.to_json_bytes
    nc.to_json_bytes = lambda: _split_waits(orig())
    return nc


_CACHE = {}


import os as _os
USE_BF16 = _os.environ.get("KERNEL_SCAN_F32", "") != "1"


def _get_nc(a_vals):
    key = (USE_BF16,) + tuple(np.round(np.asarray(a_vals, np.float64), 9))
    if key not in _CACHE:
        _CACHE[key] = build_nc(a_vals, bf16=USE_BF16)
    return _CACHE[key]


def _sel_matrix():
    s = np.zeros((2 * N_STATE, 2 * N_STATE, 128), np.float32)
    for n in range(2 * N_STATE):
        s[n, n, :] = 1.0
    return np.ascontiguousarray(s.reshape(2 * N_STATE, 2 * N_STATE * 128))


def _layernorm_c(x, gamma, beta):
    """x: (B, C, L) fp32, normalize over C."""
    x = x.astype(np.float32)
    mu = x.mean(axis=1, keepdims=True, dtype=np.float32)
    xc = x - mu
    var = np.mean(xc * xc, axis=1, keepdims=True, dtype=np.float32)
    xn = xc / np.sqrt(var + np.float32(EPS))
    return xn * gamma.astype(np.float32)[None, :, None] + beta.astype(np.float32)[None, :, None]


def kernel(**inputs):
    inp = {k: np.asarray(v) for k, v in inputs.items()}
    x = inp["x"].astype(np.float32)
    gamma, beta = inp["gamma"], inp["beta"]
    s1 = float(np.asarray(inp["s1"]).reshape(-1)[0])
    s2 = float(np.asarray(inp["s2"]).reshape(-1)[0])

    xb = x.reshape(B, C, L)
    perm = np.array([(j % GROUP) * (C // GROUP) + j // GROUP for j in range(C)])
    x1 = _layernorm_c(xb, gamma, beta)              # (B, C, L)
    x2 = _layernorm_c(xb[:, perm, :], gamma, beta)  # (B, C, L)

    a_vals = -np.exp(inp["A_log"][0].astype(np.float64))  # (16,)
    nc = _get_nc(a_vals)

    f32 = lambda a: np.ascontiguousarray(a, np.float32)
    weights = dict(
        w_in_T=f32(inp["W_in"].T),
        wx_T=f32(inp["W_x"].T.reshape(2, 128, DT_RANK + 2 * N_STATE)),
        wdt_T=f32(inp["W_dt"].T),
        wout_T=f32(inp["W_out"].T.reshape(2, 128, C)),
        convw=f32(inp["conv_w"][:, 0, :].reshape(2, 128, D_CONV)),
        convb=f32(inp["conv_b"].reshape(2, 128, 1)),
        nconvb=f32(-inp["conv_b"].reshape(2, 128, 1)),
        bdt=f32(inp["b_dt"].reshape(2, 128, 1)),
        dskip=f32(inp["D_skip"].reshape(2, 128, 1)),
        ident=np.eye(128, dtype=np.float32),
        identd=np.stack([np.diag(inp["D_skip"][:128].astype(np.float32)),
                         np.diag(inp["D_skip"][128:].astype(np.float32))]),
    )
    if not USE_BF16:
        weights["sel"] = _sel_matrix()
    in_maps = []
    for br, xbr, s in ((0, x1, s1), (1, x2, s2)):
        for b in range(B):
            m = dict(weights)
            m["xin"] = f32(xbr[b])
            m["svec"] = np.full((128, 1), s, np.float32)
            m["idents"] = (s * np.eye(128)).astype(np.float32)
            in_maps.append(m)

    res = run_bass_kernel_spmd(nc, in_maps, core_ids=list(range(8)))
    partials = [r["pout"] for r in res.results]  # (C, L) each

    if _os.environ.get("KERNEL_HOST_FINALE", "") == "1":
        out = np.empty((B, inp["W_p"].shape[0], L), np.float32)
        W_p64 = inp["W_p"].astype(np.float64)
        b_p64 = inp["b_p"].astype(np.float64)
        for b in range(B):
            xm = (partials[b].astype(np.float64) + partials[4 + b].astype(np.float64))
            mu = xm.mean(axis=0, keepdims=True)
            var = ((xm - mu) ** 2).mean(axis=0, keepdims=True)
            xmn = (xm - mu) / np.sqrt(var + EPS)
            xmn = xmn * gamma.astype(np.float64)[:, None] + beta.astype(np.float64)[:, None]
            out[b] = (W_p64 @ xmn + b_p64[:, None]).astype(np.float32)
        return out.reshape(B, -1, W_, H_)

    # phase 2 on device: 8 cores = 4 batches x 2 half-sequences
    if "nc2" not in _CACHE:
        _CACHE["nc2"] = build_nc2()
    nc2 = _CACHE["nc2"]
    W_p = inp["W_p"].astype(np.float64)
    wpg = (W_p * gamma.astype(np.float64)[None, :]).astype(np.float32)   # (out, C)
    w1 = wpg.sum(axis=1, dtype=np.float64).astype(np.float32)            # (out,)
    wbp = (inp["b_p"].astype(np.float64) + W_p @ beta.astype(np.float64)).astype(np.float32)
    w2 = dict(
        wpg_T=np.ascontiguousarray(wpg.T, np.float32),
        w1r=w1.reshape(1, C),
        wbp=wbp.reshape(1, C),
    )
    in_maps2 = []
    for b in range(B):
        for h in range(2):
            m = dict(w2)
            sl = slice(h * LH, (h + 1) * LH)
            m["pa"] = np.ascontiguousarray(partials[b][:, sl])
            m["pb"] = np.ascontiguousarray(partials[4 + b][:, sl])
            in_maps2.append(m)
    res2 = run_bass_kernel_spmd(nc2, in_maps2, core_ids=list(range(8)))
    out = np.empty((B, C, L), np.float32)
    for b in range(B):
        for h in range(2):
            out[b][:, h * LH:(h + 1) * LH] = res2.results[b * 2 + h]["fout"]
    return out.reshape(B, -1, W_, H_)



# revision 13
# speedup vs baseline: 1.1865x; 1.1865x over previous
"""Trainium2 Bass kernel for nn_DMFMLayer (Mamba-style selective-scan block).

Sharding: 2 branches x 4 batch = 8 independent scan units -> one per core.
Phase 1 (per core): full mamba chain in feature-major layout (d on
partitions, L=4096 on the free dim), processed at full sequence length:
  - causal depthwise conv fused into in_proj as 4 shifted bf16 matmuls
    with diag(conv_w_k) @ W_in pre-multiplied on host
  - selective scan via DVE tensor_tensor_scan, one (state, d-block) row
    recurrence per op; B/C broadcasts streamed via DRAM-replicate DMA
  - bt = dtu*B and p = h*C multiplies split between DVE and GPSIMD to
    balance engine occupancy
  - y accumulated over the 16 states in a full-PSUM [128, 4096] tile via
    identity matmuls
Phase 2 (8 cores = 4 batch x 2 half-seq): sum the two branch partials,
layernorm over channels (stats via PE + transposed tiny-DMA math), then
the output projection.
"""
import json
import os as _os
import sys

sys.path.insert(0, '/opt/trn_rl_repo')
import numpy as np
import ml_dtypes
import concourse.bass as bass
import concourse.mybir as mybir
from concourse.tile import TileContext
from concourse.bass_utils import run_bass_kernel_spmd

F32 = mybir.dt.float32
BF16 = mybir.dt.bfloat16
FP16 = mybir.dt.float16
AF = mybir.ActivationFunctionType
OP = mybir.AluOpType

B, C, W_, H_ = 4, 128, 64, 64
L = W_ * H_              # 4096
DI = 2 * C               # 256 (d_inner), 2 partition blocks
DT_RANK = 8
N_STATE = 16
D_CONV = 4
GROUP = 8
EPS = 1e-5
NQ = L // 512            # 8 psum-sized column chunks

BF = ml_dtypes.bfloat16

# number of the 64 bt/p multiplies that run on GPSIMD (Pool)
POOL_OPS = 30
# optional explicit assignment (list of 64 bools); overrides POOL_OPS
POOL_PATTERN = None


def _split_waits(js: bytes, max_waits: int = 1) -> bytes:
    """This walrus build allows only one sync-wait per instruction; move
    excess waits onto EventSemaphore instructions inserted just before."""
    obj = json.loads(js)

    def fix_list(lst):
        out = []
        for item in lst:
            if isinstance(item, dict) and "opcode" in item and isinstance(item.get("sync_info"), dict):
                waits = item["sync_info"].get("on_wait") or []
                if len(waits) > max_waits:
                    excess, keep = waits[:-max_waits], waits[-max_waits:]
                    for k, w in enumerate(excess):
                        out.append({
                            "engine": item.get("engine"), "ins": [], "outs": [],
                            "name": f"{item.get('name', 'I')}_sw{k}",
                            "opcode": "EventSemaphore",
                            "sync_info": {"on_update": [], "on_wait": [w]},
                        })
                    item["sync_info"]["on_wait"] = keep
            out.append(item)
        return out

    def walk(o):
        if isinstance(o, dict):
            for k, v in o.items():
                if isinstance(v, list) and any(isinstance(x, dict) and "opcode" in x for x in v):
                    o[k] = fix_list(v)
                else:
                    walk(v)
        elif isinstance(o, list):
            for v in o:
                walk(v)

    walk(obj)
    return json.dumps(obj).encode()


def build_nc(a_vals):
    """Phase 1. a_vals: 16 floats, A[n] = -exp(A_log[0, n]) baked as exp scales."""
    nc = bass.Bass()
    xin = nc.dram_tensor("xin", [C, L + D_CONV - 1], BF16, kind="ExternalInput")
    wk = nc.dram_tensor("wk", [2, D_CONV, C, 128], BF16, kind="ExternalInput")
    wz = nc.dram_tensor("wz", [2, C, 128], BF16, kind="ExternalInput")
    wx = nc.dram_tensor("wx", [2, 128, DT_RANK + 2 * N_STATE], BF16, kind="ExternalInput")
    wdt = nc.dram_tensor("wdt", [2, DT_RANK, 128], BF16, kind="ExternalInput")
    wout = nc.dram_tensor("wout", [2, 128, C], BF16, kind="ExternalInput")
    ident = nc.dram_tensor("ident", [128, 128], BF16, kind="ExternalInput")
    identd = nc.dram_tensor("identd", [2, 128, 128], BF16, kind="ExternalInput")
    idents = nc.dram_tensor("idents", [128, 128], BF16, kind="ExternalInput")
    convb = nc.dram_tensor("convb", [2, 128, 1], F32, kind="ExternalInput")
    bdt = nc.dram_tensor("bdt", [2, 128, 1], F32, kind="ExternalInput")
    xbc_dram = nc.dram_tensor("xbc_scratch", [2 * N_STATE, L], BF16, kind="Internal")
    pout = nc.dram_tensor("pout", [C, L], BF16, kind="ExternalOutput")

    with TileContext(nc) as tc:
        with (
            tc.tile_pool(name="sing", bufs=1) as sing,
            tc.tile_pool(name="work", bufs=2) as work,
        ):
            # ---- persistent weights -------------------------------------
            wk_sb = [[sing.tile([C, 128], BF16, tag=f"wk{i}{k}", name=f"wk{i}{k}")
                      for k in range(D_CONV)] for i in range(2)]
            wz_sb = [sing.tile([C, 128], BF16, tag=f"wz{i}", name=f"wz{i}") for i in range(2)]
            wx_sb = [sing.tile([128, DT_RANK + 2 * N_STATE], BF16, tag=f"wx{i}", name=f"wx{i}") for i in range(2)]
            wdt_sb = [sing.tile([DT_RANK, 128], BF16, tag=f"wdt{i}", name=f"wdt{i}") for i in range(2)]
            wout_sb = [sing.tile([128, C], BF16, tag=f"wo{i}", name=f"wo{i}") for i in range(2)]
            id_sb = sing.tile([128, 128], BF16, tag="id", name="id")
            identd_sb = [sing.tile([128, 128], BF16, tag=f"idd{i}", name=f"idd{i}") for i in range(2)]
            idents_sb = sing.tile([128, 128], BF16, tag="ids", name="ids")
            convb_sb = [sing.tile([128, 1], F32, tag=f"cb{i}", name=f"cb{i}") for i in range(2)]
            bdt_sb = [sing.tile([128, 1], F32, tag=f"bd{i}", name=f"bd{i}") for i in range(2)]
            for i in range(2):
                for k in range(D_CONV):
                    nc.sync.dma_start(out=wk_sb[i][k], in_=wk[i, k, :, :])
                nc.sync.dma_start(out=wz_sb[i], in_=wz[i, :, :])
                nc.sync.dma_start(out=wx_sb[i], in_=wx[i, :, :])
                nc.sync.dma_start(out=wdt_sb[i], in_=wdt[i, :, :])
                nc.sync.dma_start(out=wout_sb[i], in_=wout[i, :, :])
                nc.sync.dma_start(out=identd_sb[i], in_=identd[i, :, :])
                nc.sync.dma_start(out=convb_sb[i], in_=convb[i, :, :])
                nc.sync.dma_start(out=bdt_sb[i], in_=bdt[i, :, :])
            nc.sync.dma_start(out=id_sb, in_=ident[:, :])
            nc.sync.dma_start(out=idents_sb, in_=idents[:, :])

            # ---- persistent activations ---------------------------------
            x_sb = sing.tile([C, L + D_CONV - 1], BF16, tag="x", name="x")
            nc.sync.dma_start(out=x_sb, in_=xin[:, :])
            u_sb = [sing.tile([128, L], BF16, tag=f"u{i}", name=f"u{i}") for i in range(2)]
            sz_sb = [sing.tile([128, L], BF16, tag=f"sz{i}", name=f"sz{i}") for i in range(2)]
            dt_sb = [sing.tile([128, L], FP16, tag=f"dt{i}", name=f"dt{i}") for i in range(2)]
            dtu_sb = [sing.tile([128, L], BF16, tag=f"du{i}", name=f"du{i}") for i in range(2)]

            # ---- stage A: in_proj+conv, silu, x_dbl, dt -----------------
            with (
                tc.tile_pool(name="psA", bufs=1, space="PSUM") as psA,
                tc.tile_pool(name="sbA", bufs=1) as sbA,
            ):
                xdbl_sb = sbA.tile([DT_RANK + 2 * N_STATE, L], BF16, tag="xdbl", name="xdbl")
                # critical prefix first: conv+in_proj -> u -> x_dbl -> xbc/dt/dtu
                # (everything the scan pipeline waits on); z-proj deferred below
                for q in range(NQ):
                    sl = slice(q * 512, (q + 1) * 512)
                    for i in range(2):
                        pu = psA.tile([128, 512], F32, tag="mma", name="pu", bufs=3)
                        for k in range(D_CONV):
                            nc.tensor.matmul(pu, wk_sb[i][k], x_sb[:, q * 512 + k: q * 512 + k + 512],
                                             start=(k == 0), stop=(k == D_CONV - 1))
                        nc.scalar.activation(u_sb[i][:, sl], pu, AF.Silu, bias=convb_sb[i][:, 0:1])
                    pxd = psA.tile([DT_RANK + 2 * N_STATE, 512], F32, tag="mmx", name="pxd", bufs=2)
                    nc.tensor.matmul(pxd, wx_sb[0], u_sb[0][:, sl], start=True, stop=False)
                    nc.tensor.matmul(pxd, wx_sb[1], u_sb[1][:, sl], start=False, stop=True)
                    nc.scalar.copy(xdbl_sb[:, sl], pxd)
                    nc.sync.dma_start(out=xbc_dram[:, sl], in_=xdbl_sb[DT_RANK:, sl])
                    for i in range(2):
                        pdt = psA.tile([128, 512], F32, tag="mma", name="pdt", bufs=3)
                        nc.tensor.matmul(pdt, wdt_sb[i], xdbl_sb[0:DT_RANK, sl], start=True, stop=True)
                        edt = sbA.tile([128, 512], F32, tag="edt", name="edt", bufs=2)
                        nc.scalar.activation(edt, pdt, AF.Exp, bias=bdt_sb[i][:, 0:1])
                        nc.scalar.activation(dt_sb[i][:, sl], edt, AF.Ln, bias=1.0)
                for i in range(2):
                    nc.vector.tensor_tensor(out=dtu_sb[i], in0=dt_sb[i], in1=u_sb[i], op=OP.mult)
                # z-projection off the critical prefix; sz only needed at g
                for q in range(NQ):
                    sl = slice(q * 512, (q + 1) * 512)
                    for i in range(2):
                        pz = psA.tile([128, 512], F32, tag="mma", name="pz", bufs=3)
                        nc.tensor.matmul(pz, wz_sb[i], x_sb[:, q * 512 + 3: q * 512 + 3 + 512],
                                         start=True, stop=True)
                        nc.scalar.activation(sz_sb[i][:, sl], pz, AF.Silu)

            # ---- stage B: selective scan, y accumulation ----------------
            # software-pipelined: DMAs at lag 0, dA/bt at lag 1, scan/p/mm at lag 2
            iters = [(i, n) for i in range(2) for n in range(N_STATE)]
            NI = len(iters)
            # op-level DVE/Pool split for the bt/p multiplies
            on_pool = [(j * POOL_OPS) // (2 * NI) != ((j + 1) * POOL_OPS) // (2 * NI)
                       for j in range(2 * NI)]
            py = [None, None]
            tiles = {}
            with (
                tc.tile_pool(name="psB", bufs=1, space="PSUM") as psB,
                tc.tile_pool(name="sbB", bufs=1) as sbB,
            ):
                for step in range(NI + 2):
                    if step < NI:
                        idx = step
                        i, n = iters[idx]
                        d = tiles.setdefault(idx, {})
                        d["brep"] = sbB.tile([128, L], BF16, tag="brep", name="brep", bufs=3)
                        bsrc = xbc_dram[n, 0]
                        nc.sync.dma_start(out=d["brep"], in_=bass.AP(
                            tensor=bsrc.tensor, offset=bsrc.offset, ap=[[0, 128], [1, L]]))
                        d["crep"] = sbB.tile([128, L], BF16, tag="crep", name="crep", bufs=3)
                        csrc = xbc_dram[N_STATE + n, 0]
                        nc.scalar.dma_start(out=d["crep"], in_=bass.AP(
                            tensor=csrc.tensor, offset=csrc.offset, ap=[[0, 128], [1, L]]))
                    if 0 <= step - 1 < NI:
                        idx = step - 1
                        i, n = iters[idx]
                        d = tiles[idx]
                        d["dA"] = sbB.tile([128, L], BF16, tag="dA", name="dA", bufs=3)
                        nc.scalar.activation(d["dA"], dt_sb[i], AF.Exp, scale=float(a_vals[n]))
                        d["bt"] = sbB.tile([128, L], BF16, tag="btp", name="bt", bufs=4)
                        eng = nc.gpsimd if on_pool[2 * idx] else nc.vector
                        eng.tensor_tensor(out=d["bt"], in0=dtu_sb[i], in1=d["brep"], op=OP.mult)
                    if 0 <= step - 2 < NI:
                        idx = step - 2
                        i, n = iters[idx]
                        d = tiles.pop(idx)
                        if n == 0:
                            py[i] = psB.tile([128, L], F32, tag="py", name=f"py{i}")
                        h = sbB.tile([128, L], BF16, tag="h", name="h", bufs=2)
                        nc.vector.tensor_tensor_scan(out=h, data0=d["dA"], data1=d["bt"], initial=0.0,
                                                     op0=OP.mult, op1=OP.add)
                        p = sbB.tile([128, L], BF16, tag="btp", name="p", bufs=4)
                        eng = nc.gpsimd if on_pool[2 * idx + 1] else nc.vector
                        eng.tensor_tensor(out=p, in0=h, in1=d["crep"], op=OP.mult)
                        for q in range(NQ):
                            nc.tensor.matmul(py[i][:, q * 512:(q + 1) * 512], id_sb, p[:, q * 512:(q + 1) * 512],
                                             start=(n == 0), stop=False, skip_group_check=True)
                        if n == N_STATE - 1:
                            # y += u * D_skip, then g = silu(z) * y (g overwrites the dead u tile)
                            for q in range(NQ):
                                nc.tensor.matmul(py[i][:, q * 512:(q + 1) * 512], identd_sb[i],
                                                 u_sb[i][:, q * 512:(q + 1) * 512],
                                                 start=False, stop=True, skip_group_check=True)
                            for c2 in range(2):
                                c2sl = slice(c2 * 2048, (c2 + 1) * 2048)
                                nc.vector.tensor_tensor(out=u_sb[i][:, c2sl], in0=sz_sb[i][:, c2sl],
                                                        in1=py[i][:, c2sl], op=OP.mult)

                # ---- stage C: out_proj + s*x skip -----------------------
                po = psB.tile([128, L], F32, tag="py", name="po")
                for q in range(NQ):
                    sl = slice(q * 512, (q + 1) * 512)
                    pq = po[:, sl]
                    nc.tensor.matmul(pq, wout_sb[0], u_sb[0][:, sl], start=True, stop=False, skip_group_check=True)
                    nc.tensor.matmul(pq, wout_sb[1], u_sb[1][:, sl], start=False, stop=False, skip_group_check=True)
                    nc.tensor.matmul(pq, idents_sb, x_sb[:, q * 512 + 3: q * 512 + 3 + 512],
                                     start=False, stop=True, skip_group_check=True)
                    nc.scalar.copy(sz_sb[0][:, sl], pq)
                    nc.sync.dma_start(out=pout[:, sl], in_=sz_sb[0][:, sl])

    orig = nc.to_json_bytes
    nc.to_json_bytes = lambda: _split_waits(orig())
    return nc


LH = L // 2  # 2048, half-sequence per finale core


def build_nc2():
    """Phase 2: xm = pa+pb; LN over C (partition dim); out = W_p@xmn + b_p."""
    nc = bass.Bass()
    pa = nc.dram_tensor("pa", [C, LH], BF16, kind="ExternalInput")
    pb = nc.dram_tensor("pb", [C, LH], BF16, kind="ExternalInput")
    wpg_T = nc.dram_tensor("wpg_T", [C, C], BF16, kind="ExternalInput")
    w1r = nc.dram_tensor("w1r", [1, C], BF16, kind="ExternalInput")     # -(Wpg @ ones), applied via ms = mu*rs
    wbp = nc.dram_tensor("wbp", [1, C], BF16, kind="ExternalInput")
    rows_dram = nc.dram_tensor("rows_scratch", [2, LH], BF16, kind="Internal")
    fout = nc.dram_tensor("fout", [C, LH], F32, kind="ExternalOutput")
    NT = LH // 128  # 16

    with TileContext(nc) as tc:
        with (
            tc.tile_pool(name="sgl", bufs=1) as sgl,
            tc.tile_pool(name="wrk", bufs=2) as wrk,
            tc.tile_pool(name="ps", bufs=1, space="PSUM") as ps,
        ):
            wpg_sb = sgl.tile([C, C], BF16, tag="wpg", name="wpg")
            nc.sync.dma_start(out=wpg_sb, in_=wpg_T[:, :])
            w1_sb = sgl.tile([1, C], BF16, tag="w1", name="w1")
            nc.sync.dma_start(out=w1_sb, in_=w1r[:, :])
            wbp_sb = sgl.tile([1, C], BF16, tag="wbp", name="wbp")
            nc.sync.dma_start(out=wbp_sb, in_=wbp[:, :])
            ones_col = sgl.tile([C, 1], BF16, tag="onesc", name="onesc")
            nc.vector.memset(ones_col, 1.0)
            ones_row = sgl.tile([1, LH], BF16, tag="onesr", name="onesr")
            nc.vector.memset(ones_row, 1.0)
            ones_p1 = sgl.tile([1, 128], BF16, tag="onesp", name="onesp")
            nc.vector.memset(ones_p1, 1.0)
            eps_col = sgl.tile([128, 1], F32, tag="epsc", name="epsc")
            nc.vector.memset(eps_col, EPS)

            pa_sb = sgl.tile([C, LH], BF16, tag="pa", name="pa")
            nc.sync.dma_start(out=pa_sb, in_=pa[:, :])
            pb_sb = sgl.tile([C, LH], BF16, tag="pb", name="pb")
            nc.sync.dma_start(out=pb_sb, in_=pb[:, :])
            xm = sgl.tile([C, LH], BF16, tag="xm", name="xm")
            nc.vector.tensor_tensor(out=xm, in0=pa_sb, in1=pb_sb, op=OP.add)
            xsq = sgl.tile([C, LH], BF16, tag="xsq", name="xsq")
            nc.scalar.activation(xsq, xm, AF.Square)

            # column stats via PE, transposed to [128, NT] via tiny DMAs
            s1t = sgl.tile([128, NT], F32, tag="s1t", name="s1t")
            s2t = sgl.tile([128, NT], F32, tag="s2t", name="s2t")
            for q in range(LH // 512):
                sl = slice(q * 512, (q + 1) * 512)
                s1p = ps.tile([1, 512], F32, tag="s1", name="s1", bufs=2)
                nc.tensor.matmul(s1p, ones_col, xm[:, sl], start=True, stop=True)
                s2p = ps.tile([1, 512], F32, tag="s2", name="s2", bufs=2)
                nc.tensor.matmul(s2p, ones_col, xsq[:, sl], start=True, stop=True)
                s1q = wrk.tile([1, 512], F32, tag="s1q", name="s1q")
                nc.vector.tensor_copy(s1q, s1p)
                s2q = wrk.tile([1, 512], F32, tag="s2q", name="s2q")
                nc.vector.tensor_copy(s2q, s2p)
                nc.sync.dma_start(out=s1t[:, q * 4:(q + 1) * 4], in_=s1q)
                nc.sync.dma_start(out=s2t[:, q * 4:(q + 1) * 4], in_=s2q)

            mu = sgl.tile([128, NT], F32, tag="mu", name="mu")
            nc.vector.tensor_scalar(mu, s1t, 1.0 / C, None, op0=OP.mult)
            m2 = sgl.tile([128, NT], F32, tag="m2", name="m2")
            nc.vector.tensor_scalar(m2, s2t, 1.0 / C, None, op0=OP.mult)
            musq = sgl.tile([128, NT], F32, tag="musq", name="musq")
            nc.vector.tensor_tensor(out=musq, in0=mu, in1=mu, op=OP.mult)
            nc.vector.tensor_tensor(out=m2, in0=m2, in1=musq, op=OP.subtract)
            sd = sgl.tile([128, NT], F32, tag="sd", name="sd")
            nc.scalar.activation(sd, m2, AF.Sqrt, bias=eps_col[:, 0:1])
            rs = sgl.tile([128, NT], F32, tag="rs", name="rs")
            nc.vector.reciprocal(rs, sd)
            ms = sgl.tile([128, NT], F32, tag="ms", name="ms")
            nc.vector.tensor_tensor(out=ms, in0=mu, in1=rs, op=OP.mult)
            rs_bf = sgl.tile([128, NT], BF16, tag="rsb", name="rsb")
            nc.vector.tensor_copy(rs_bf, rs)
            ms_bf = sgl.tile([128, NT], BF16, tag="msb", name="msb")
            nc.vector.tensor_copy(ms_bf, ms)
            # inverse of the stats transpose: rows[512q + 4p + j] = t[p, 4q + j]
            for row_idx, t_bf in ((0, rs_bf), (1, ms_bf)):
                dst = rows_dram[row_idx, 0]
                nc.sync.dma_start(
                    out=bass.AP(tensor=dst.tensor, offset=dst.offset,
                                ap=[[4, 128], [512, NT // 4], [1, 4]]),
                    in_=bass.AP(tensor=t_bf.tensor, offset=t_bf.offset,
                                ap=[[t_bf.ap[0][0], 128], [4, NT // 4], [1, 4]]))
            rs_row = sgl.tile([1, LH], BF16, tag="rsr", name="rsr")
            nc.sync.dma_start(out=rs_row, in_=rows_dram[0, :])
            ms_row = sgl.tile([1, LH], BF16, tag="msr", name="msr")
            nc.sync.dma_start(out=ms_row, in_=rows_dram[1, :])

            out_f = sgl.tile([C, LH], F32, tag="of", name="of")
            for q in range(LH // 512):
                sl = slice(q * 512, (q + 1) * 512)
                prep = ps.tile([128, 512], F32, tag="prep", name="prep", bufs=2)
                nc.tensor.matmul(prep, ones_p1, rs_row[:, sl], start=True, stop=True)
                xms = wrk.tile([128, 512], BF16, tag="xms", name="xms")
                nc.vector.tensor_tensor(out=xms, in0=xm[:, sl], in1=prep, op=OP.mult)
                po2 = ps.tile([128, 512], F32, tag="po2", name="po2", bufs=2)
                nc.tensor.matmul(po2, wpg_sb, xms, start=True, stop=False)
                nc.tensor.matmul(po2, w1_sb, ms_row[:, sl], start=False, stop=False, skip_group_check=True)
                nc.tensor.matmul(po2, wbp_sb, ones_row[:, sl], start=False, stop=True, skip_group_check=True)
                nc.scalar.copy(out_f[:, sl], po2)
            nc.sync.dma_start(out=fout[:, :], in_=out_f)

    orig = nc.to_json_bytes
    nc.to_json_bytes = lambda: _split_waits(orig())
    return nc


_CACHE = {}


def _get_nc(a_vals):
    key = tuple(np.round(np.asarray(a_vals, np.float64), 9))
    if key not in _CACHE:
        _CACHE[key] = build_nc(key)
    return _CACHE[key]


def _layernorm_c(x, gamma, beta):
    """x: (B, C, L) fp32, normalize over C."""
    x = x.astype(np.float32)
    mu = x.mean(axis=1, keepdims=True, dtype=np.float32)
    xc = x - mu
    var = np.mean(xc * xc, axis=1, keepdims=True, dtype=np.float32)
    xn = xc / np.sqrt(var + np.float32(EPS))
    return xn * gamma.astype(np.float32)[None, :, None] + beta.astype(np.float32)[None, :, None]


def kernel(**inputs):
    inp = {k: np.asarray(v) for k, v in inputs.items()}
    x = inp["x"].astype(np.float32)
    gamma, beta = inp["gamma"], inp["beta"]
    s1 = float(np.asarray(inp["s1"]).reshape(-1)[0])
    s2 = float(np.asarray(inp["s2"]).reshape(-1)[0])

    xb = x.reshape(B, C, L)
    perm = np.array([(j % GROUP) * (C // GROUP) + j // GROUP for j in range(C)])
    x1 = _layernorm_c(xb, gamma, beta)              # (B, C, L)
    x2 = _layernorm_c(xb[:, perm, :], gamma, beta)  # (B, C, L)

    a_vals = -np.exp(inp["A_log"][0].astype(np.float64))  # (16,)
    nc = _get_nc(a_vals)

    bf = lambda a: np.ascontiguousarray(np.asarray(a, np.float32).astype(BF))
    f32 = lambda a: np.ascontiguousarray(a, np.float32)
    W_in = inp["W_in"].astype(np.float64)           # (512, 128)
    conv_w = inp["conv_w"][:, 0, :].astype(np.float64)  # (256, 4)
    # conv fused into in_proj: wk[i,k] = (diag(conv_w[i-block,k]) @ W_in_block_i).T
    wk = np.empty((2, D_CONV, C, 128), np.float64)
    for i in range(2):
        blk = W_in[i * 128:(i + 1) * 128, :]        # (128, C)
        for k in range(D_CONV):
            wk[i, k] = (conv_w[i * 128:(i + 1) * 128, k][:, None] * blk).T
    wz = np.stack([W_in[DI + i * 128: DI + (i + 1) * 128, :].T for i in range(2)])
    W_x = inp["W_x"].astype(np.float64)             # (40, 256)
    wx = np.stack([W_x[:, i * 128:(i + 1) * 128].T for i in range(2)])
    W_dt = inp["W_dt"].astype(np.float64)           # (256, 8)
    wdt = np.stack([W_dt[i * 128:(i + 1) * 128, :].T for i in range(2)])
    W_out = inp["W_out"].astype(np.float64)         # (128, 256)
    wout = np.stack([W_out[:, i * 128:(i + 1) * 128].T for i in range(2)])
    D_skip = inp["D_skip"].astype(np.float64)

    weights = dict(
        wk=bf(wk), wz=bf(wz), wx=bf(wx), wdt=bf(wdt), wout=bf(wout),
        ident=bf(np.eye(128)),
        identd=bf(np.stack([np.diag(D_skip[:128]), np.diag(D_skip[128:])])),
        convb=f32(inp["conv_b"].reshape(2, 128, 1)),
        bdt=f32(inp["b_dt"].reshape(2, 128, 1)),
    )
    pad = np.zeros((B, C, D_CONV - 1), np.float32)
    in_maps = []
    for br, xbr, s in ((0, x1, s1), (1, x2, s2)):
        for b in range(B):
            m = dict(weights)
            m["xin"] = bf(np.concatenate([pad[b], xbr[b]], axis=1))
            m["idents"] = bf(s * np.eye(128))
            in_maps.append(m)

    res = run_bass_kernel_spmd(nc, in_maps, core_ids=list(range(8)))
    partials = [r["pout"] for r in res.results]  # (C, L) bf16 each

    # phase 2 on device: 8 cores = 4 batches x 2 half-sequences
    if "nc2" not in _CACHE:
        _CACHE["nc2"] = build_nc2()
    nc2 = _CACHE["nc2"]
    W_p = inp["W_p"].astype(np.float64)
    wpg = W_p * gamma.astype(np.float64)[None, :]                        # (out, C)
    w1 = -wpg.sum(axis=1)                                                # (out,)
    wbp = inp["b_p"].astype(np.float64) + W_p @ beta.astype(np.float64)
    w2 = dict(
        wpg_T=bf(wpg.T),
        w1r=bf(w1.reshape(1, C)),
        wbp=bf(wbp.reshape(1, C)),
    )
    in_maps2 = []
    for b in range(B):
        for h in range(2):
            m = dict(w2)
            sl = slice(h * LH, (h + 1) * LH)
            m["pa"] = np.ascontiguousarray(partials[b][:, sl])
            m["pb"] = np.ascontiguousarray(partials[4 + b][:, sl])
            in_maps2.append(m)
    res2 = run_bass_kernel_spmd(nc2, in_maps2, core_ids=list(range(8)))
    out = np.empty((B, C, L), np.float32)
    for b in range(B):
        for h in range(2):
            out[b][:, h * LH:(h + 1) * LH] = res2.results[b * 2 + h]["fout"]
    return out.reshape(B, -1, W_, H_)


# revision 19
# speedup vs baseline: 1.2924x; 1.0893x over previous
"""Trainium2 Bass kernel for nn_DMFMLayer (Mamba-style selective-scan block).

Sharding: 2 branches x 4 batch = 8 independent scan units -> one per core.
Phase 1 (per core): full mamba chain in feature-major layout (d on
partitions, L=4096 on the free dim), processed at full sequence length:
  - causal depthwise conv fused into in_proj as 4 shifted bf16 matmuls
    with diag(conv_w_k) @ W_in pre-multiplied on host
  - selective scan via DVE tensor_tensor_scan, one (state, d-block) row
    recurrence per op; B/C broadcasts streamed via DRAM-replicate DMA
  - bt = dtu*B and p = h*C multiplies split between DVE and GPSIMD to
    balance engine occupancy
  - y accumulated over the 16 states in a full-PSUM [128, 4096] tile via
    identity matmuls
Phase 2 (8 cores = 4 batch x 2 half-seq): sum the two branch partials,
layernorm over channels (stats via PE + transposed tiny-DMA math), then
the output projection.
"""
import json
import os as _os
import sys

sys.path.insert(0, '/opt/trn_rl_repo')
import numpy as np
import ml_dtypes
import concourse.bass as bass
import concourse.mybir as mybir
from concourse.tile import TileContext
from concourse.bass_utils import run_bass_kernel_spmd

F32 = mybir.dt.float32
BF16 = mybir.dt.bfloat16
FP16 = mybir.dt.float16
AF = mybir.ActivationFunctionType
OP = mybir.AluOpType

B, C, W_, H_ = 4, 128, 64, 64
L = W_ * H_              # 4096
DI = 2 * C               # 256 (d_inner), 2 partition blocks
DT_RANK = 8
N_STATE = 16
D_CONV = 4
GROUP = 8
EPS = 1e-5
NQ = L // 512            # 8 psum-sized column chunks

BF = ml_dtypes.bfloat16

# number of the 64 bt/p multiplies that run on GPSIMD (Pool)
POOL_OPS = 30
# op-level DVE/Pool assignment for the 64 bt/p multiplies (op 2k = bt of
# pipeline iteration k, op 2k+1 = its p): all bt on Pool except the last
# two drain iterations; all p on DVE. Balances Pool ~249us vs DVE ~235us
# with Pool fed only by DMA (no cross-engine head-of-line stalls).
POOL_PATTERN = [(j % 2 == 0 and j < 60) for j in range(64)]


def _split_waits(js: bytes, max_waits: int = 1) -> bytes:
    """This walrus build allows only one sync-wait per instruction; move
    excess waits onto EventSemaphore instructions inserted just before."""
    obj = json.loads(js)

    def fix_list(lst):
        out = []
        for item in lst:
            if isinstance(item, dict) and "opcode" in item and isinstance(item.get("sync_info"), dict):
                waits = item["sync_info"].get("on_wait") or []
                if len(waits) > max_waits:
                    excess, keep = waits[:-max_waits], waits[-max_waits:]
                    for k, w in enumerate(excess):
                        out.append({
                            "engine": item.get("engine"), "ins": [], "outs": [],
                            "name": f"{item.get('name', 'I')}_sw{k}",
                            "opcode": "EventSemaphore",
                            "sync_info": {"on_update": [], "on_wait": [w]},
                        })
                    item["sync_info"]["on_wait"] = keep
            out.append(item)
        return out

    def walk(o):
        if isinstance(o, dict):
            for k, v in o.items():
                if isinstance(v, list) and any(isinstance(x, dict) and "opcode" in x for x in v):
                    o[k] = fix_list(v)
                else:
                    walk(v)
        elif isinstance(o, list):
            for v in o:
                walk(v)

    walk(obj)
    return json.dumps(obj).encode()


def build_nc(a_vals):
    """Phase 1. a_vals: 16 floats, A[n] = -exp(A_log[0, n]) baked as exp scales."""
    nc = bass.Bass()
    xin = nc.dram_tensor("xin", [C, L + D_CONV - 1], BF16, kind="ExternalInput")
    # all bf16 weights column-packed into one tensor (one DMA at startup):
    # wk(8) wz(2) wout(2) ident identd(2) idents wdt(2) | wx(2 x 40 cols)
    WCOLS = 16 * 128 + 2 * 128 + 2 * (DT_RANK + 2 * N_STATE)
    wpack = nc.dram_tensor("wpack", [C, WCOLS], BF16, kind="ExternalInput")
    bpack = nc.dram_tensor("bpack", [C, 4], F32, kind="ExternalInput")
    xbc_dram = nc.dram_tensor("xbc_scratch", [2 * N_STATE, L], BF16, kind="Internal")
    pout = nc.dram_tensor("pout", [C, L], BF16, kind="ExternalOutput")

    with TileContext(nc) as tc:
        with (
            tc.tile_pool(name="sing", bufs=1) as sing,
            tc.tile_pool(name="work", bufs=2) as work,
        ):
            # ---- persistent activations (x first: the conv chain gates B)
            x_sb = sing.tile([C, L + D_CONV - 1], BF16, tag="x", name="x")
            nc.sync.dma_start(out=x_sb, in_=xin[:, :])

            # ---- persistent weights: one packed DMA ----------------------
            WCOLS = 16 * 128 + 2 * 128 + 2 * (DT_RANK + 2 * N_STATE)
            wp_sb = sing.tile([C, WCOLS], BF16, tag="wpack", name="wpack")
            nc.scalar.dma_start(out=wp_sb, in_=wpack[:, :])
            bp_sb = sing.tile([C, 4], F32, tag="bpack", name="bpack")
            nc.scalar.dma_start(out=bp_sb, in_=bpack[:, :])
            blk = lambda j: wp_sb[:, j * 128:(j + 1) * 128]
            wk_sb = [[blk(i * D_CONV + k) for k in range(D_CONV)] for i in range(2)]
            wz_sb = [blk(8 + i) for i in range(2)]
            wout_sb = [blk(10 + i) for i in range(2)]
            id_sb = blk(12)
            identd_sb = [blk(13 + i) for i in range(2)]
            idents_sb = blk(15)
            wdt_sb = [wp_sb[0:DT_RANK, (16 + i) * 128:(16 + i) * 128 + 128] for i in range(2)]
            NXD = DT_RANK + 2 * N_STATE
            wx_sb = [wp_sb[:, 18 * 128 + i * NXD: 18 * 128 + (i + 1) * NXD] for i in range(2)]
            convb_sb = [bp_sb[:, i:i + 1] for i in range(2)]
            bdt_sb = [bp_sb[:, 2 + i:3 + i] for i in range(2)]
            u_sb = [sing.tile([128, L], BF16, tag=f"u{i}", name=f"u{i}") for i in range(2)]
            sz_sb = [sing.tile([128, L], BF16, tag=f"sz{i}", name=f"sz{i}") for i in range(2)]
            dt_sb = [sing.tile([128, L], FP16, tag=f"dt{i}", name=f"dt{i}") for i in range(2)]
            dtu_sb = [sing.tile([128, L], BF16, tag=f"du{i}", name=f"du{i}") for i in range(2)]

            # ---- stage A: in_proj+conv, silu, x_dbl, dt -----------------
            with (
                tc.tile_pool(name="psA", bufs=1, space="PSUM") as psA,
                tc.tile_pool(name="sbA", bufs=1) as sbA,
            ):
                xdbl_sb = sbA.tile([DT_RANK + 2 * N_STATE, L], BF16, tag="xdbl", name="xdbl")
                # critical prefix first: conv+in_proj -> u -> x_dbl -> xbc/dt/dtu
                # (everything the scan pipeline waits on); z-proj deferred below
                for q in range(NQ):
                    sl = slice(q * 512, (q + 1) * 512)
                    for i in range(2):
                        pu = psA.tile([128, 512], F32, tag="mma", name="pu", bufs=3)
                        for k in range(D_CONV):
                            nc.tensor.matmul(pu, wk_sb[i][k], x_sb[:, q * 512 + k: q * 512 + k + 512],
                                             start=(k == 0), stop=(k == D_CONV - 1))
                        nc.scalar.activation(u_sb[i][:, sl], pu, AF.Silu, bias=convb_sb[i])
                    pxd = psA.tile([DT_RANK + 2 * N_STATE, 512], F32, tag="mmx", name="pxd", bufs=2)
                    nc.tensor.matmul(pxd, wx_sb[0], u_sb[0][:, sl], start=True, stop=False)
                    nc.tensor.matmul(pxd, wx_sb[1], u_sb[1][:, sl], start=False, stop=True)
                    nc.scalar.copy(xdbl_sb[:, sl], pxd)
                    nc.sync.dma_start(out=xbc_dram[:, sl], in_=xdbl_sb[DT_RANK:, sl])
                    for i in range(2):
                        pdt = psA.tile([128, 512], F32, tag="mma", name="pdt", bufs=3)
                        nc.tensor.matmul(pdt, wdt_sb[i], xdbl_sb[0:DT_RANK, sl], start=True, stop=True)
                        edt = sbA.tile([128, 512], F32, tag="edt", name="edt", bufs=2)
                        nc.scalar.activation(edt, pdt, AF.Exp, bias=bdt_sb[i])
                        nc.scalar.activation(dt_sb[i][:, sl], edt, AF.Ln, bias=1.0)
                for i in range(2):
                    nc.vector.tensor_tensor(out=dtu_sb[i], in0=dt_sb[i], in1=u_sb[i], op=OP.mult)
                # z-projection off the critical prefix; sz only needed at g
                for q in range(NQ):
                    sl = slice(q * 512, (q + 1) * 512)
                    for i in range(2):
                        pz = psA.tile([128, 512], F32, tag="mma", name="pz", bufs=3)
                        nc.tensor.matmul(pz, wz_sb[i], x_sb[:, q * 512 + 3: q * 512 + 3 + 512],
                                         start=True, stop=True)
                        nc.scalar.activation(sz_sb[i][:, sl], pz, AF.Silu)

            # ---- stage B: selective scan, y accumulation ----------------
            # software-pipelined: DMAs at lag 0, dA/bt at lag 1, scan/p/mm at lag 2
            iters = [(i, n) for i in range(2) for n in range(N_STATE)]
            NI = len(iters)
            # op-level DVE/Pool split for the bt/p multiplies
            if POOL_PATTERN is not None:
                on_pool = list(POOL_PATTERN)
            else:
                on_pool = [(j * POOL_OPS) // (2 * NI) != ((j + 1) * POOL_OPS) // (2 * NI)
                           for j in range(2 * NI)]
            py = [None, None]
            tiles = {}
            with (
                tc.tile_pool(name="psB", bufs=1, space="PSUM") as psB,
                tc.tile_pool(name="sbB", bufs=1) as sbB,
            ):
                for step in range(NI + 2):
                    if step < NI:
                        idx = step
                        i, n = iters[idx]
                        d = tiles.setdefault(idx, {})
                        d["brep"] = sbB.tile([128, L], BF16, tag="brep", name="brep", bufs=3)
                        bsrc = xbc_dram[n, 0]
                        nc.sync.dma_start(out=d["brep"], in_=bass.AP(
                            tensor=bsrc.tensor, offset=bsrc.offset, ap=[[0, 128], [1, L]]))
                        d["crep"] = sbB.tile([128, L], BF16, tag="crep", name="crep", bufs=3)
                        csrc = xbc_dram[N_STATE + n, 0]
                        nc.scalar.dma_start(out=d["crep"], in_=bass.AP(
                            tensor=csrc.tensor, offset=csrc.offset, ap=[[0, 128], [1, L]]))
                    if 0 <= step - 1 < NI:
                        idx = step - 1
                        i, n = iters[idx]
                        d = tiles[idx]
                        d["dA"] = sbB.tile([128, L], BF16, tag="dA", name="dA", bufs=3)
                        nc.scalar.activation(d["dA"], dt_sb[i], AF.Exp, scale=float(a_vals[n]))
                        d["bt"] = sbB.tile([128, L], BF16, tag="btp", name="bt", bufs=4)
                        eng = nc.gpsimd if on_pool[2 * idx] else nc.vector
                        eng.tensor_tensor(out=d["bt"], in0=dtu_sb[i], in1=d["brep"], op=OP.mult)
                    if 0 <= step - 2 < NI:
                        idx = step - 2
                        i, n = iters[idx]
                        d = tiles.pop(idx)
                        if n == 0:
                            py[i] = psB.tile([128, L], F32, tag="py", name=f"py{i}")
                        h = sbB.tile([128, L], BF16, tag="h", name="h", bufs=2)
                        nc.vector.tensor_tensor_scan(out=h, data0=d["dA"], data1=d["bt"], initial=0.0,
                                                     op0=OP.mult, op1=OP.add)
                        p = sbB.tile([128, L], BF16, tag="btp", name="p", bufs=4)
                        eng = nc.gpsimd if on_pool[2 * idx + 1] else nc.vector
                        eng.tensor_tensor(out=p, in0=h, in1=d["crep"], op=OP.mult)
                        for q in range(NQ):
                            nc.tensor.matmul(py[i][:, q * 512:(q + 1) * 512], id_sb, p[:, q * 512:(q + 1) * 512],
                                             start=(n == 0), stop=False, skip_group_check=True)
                        if n == N_STATE - 1:
                            # y += u * D_skip, then g = silu(z) * y (g overwrites the dead u tile)
                            for q in range(NQ):
                                nc.tensor.matmul(py[i][:, q * 512:(q + 1) * 512], identd_sb[i],
                                                 u_sb[i][:, q * 512:(q + 1) * 512],
                                                 start=False, stop=True, skip_group_check=True)
                            for c2 in range(2):
                                c2sl = slice(c2 * 2048, (c2 + 1) * 2048)
                                nc.vector.tensor_tensor(out=u_sb[i][:, c2sl], in0=sz_sb[i][:, c2sl],
                                                        in1=py[i][:, c2sl], op=OP.mult)

                # ---- stage C: out_proj + s*x skip -----------------------
                po = psB.tile([128, L], F32, tag="py", name="po")
                for q in range(NQ):
                    sl = slice(q * 512, (q + 1) * 512)
                    pq = po[:, sl]
                    nc.tensor.matmul(pq, wout_sb[0], u_sb[0][:, sl], start=True, stop=False, skip_group_check=True)
                    nc.tensor.matmul(pq, wout_sb[1], u_sb[1][:, sl], start=False, stop=False, skip_group_check=True)
                    nc.tensor.matmul(pq, idents_sb, x_sb[:, q * 512 + 3: q * 512 + 3 + 512],
                                     start=False, stop=True, skip_group_check=True)
                    nc.scalar.copy(sz_sb[0][:, sl], pq)
                    nc.sync.dma_start(out=pout[:, sl], in_=sz_sb[0][:, sl])

    orig = nc.to_json_bytes
    nc.to_json_bytes = lambda: _split_waits(orig())
    return nc


LH = L // 2  # 2048, half-sequence per finale core


def build_nc2():
    """Phase 2: xm = pa+pb; LN over C (partition dim); out = W_p@xmn + b_p.
    All LN stats stay on [1, 512] row tiles (free-dim ops are cheap at this
    width and avoid any transpose DMA roundtrip); fully pipelined per
    512-column chunk."""
    nc = bass.Bass()
    pa = nc.dram_tensor("pa", [C, LH], BF16, kind="ExternalInput")
    pb = nc.dram_tensor("pb", [C, LH], BF16, kind="ExternalInput")
    wpg_T = nc.dram_tensor("wpg_T", [C, C], BF16, kind="ExternalInput")
    w1r = nc.dram_tensor("w1r", [1, C], BF16, kind="ExternalInput")     # -(Wpg @ ones), applied via ms = mu*rs
    wbp = nc.dram_tensor("wbp", [1, C], BF16, kind="ExternalInput")
    fout = nc.dram_tensor("fout", [C, LH], F32, kind="ExternalOutput")

    with TileContext(nc) as tc:
        with (
            tc.tile_pool(name="sgl", bufs=1) as sgl,
            tc.tile_pool(name="wrk", bufs=3) as wrk,
            tc.tile_pool(name="ps", bufs=1, space="PSUM") as ps,
        ):
            pa_sb = sgl.tile([C, LH], BF16, tag="pa", name="pa")
            nc.sync.dma_start(out=pa_sb, in_=pa[:, :])
            pb_sb = sgl.tile([C, LH], BF16, tag="pb", name="pb")
            nc.scalar.dma_start(out=pb_sb, in_=pb[:, :])
            wpg_sb = sgl.tile([C, C], BF16, tag="wpg", name="wpg")
            nc.sync.dma_start(out=wpg_sb, in_=wpg_T[:, :])
            w1_sb = sgl.tile([1, C], BF16, tag="w1", name="w1")
            nc.sync.dma_start(out=w1_sb, in_=w1r[:, :])
            wbp_sb = sgl.tile([1, C], BF16, tag="wbp", name="wbp")
            nc.sync.dma_start(out=wbp_sb, in_=wbp[:, :])
            ones_col = sgl.tile([C, 1], BF16, tag="onesc", name="onesc")
            nc.vector.memset(ones_col, 1.0)
            ones_row = sgl.tile([1, LH], BF16, tag="onesr", name="onesr")
            nc.vector.memset(ones_row, 1.0)
            ones_p1 = sgl.tile([1, 128], BF16, tag="onesp", name="onesp")
            nc.vector.memset(ones_p1, 1.0)
            eps_row = sgl.tile([1, 1], F32, tag="epsr", name="epsr")
            nc.vector.memset(eps_row, EPS)

            xm = sgl.tile([C, LH], BF16, tag="xm", name="xm")
            out_f = sgl.tile([C, LH], F32, tag="of", name="of")
            for q in range(LH // 512):
                sl = slice(q * 512, (q + 1) * 512)
                nc.vector.tensor_tensor(out=xm[:, sl], in0=pa_sb[:, sl], in1=pb_sb[:, sl], op=OP.add)
                xsq = wrk.tile([C, 512], BF16, tag="xsq", name="xsq")
                nc.scalar.activation(xsq, xm[:, sl], AF.Square)
                s1p = ps.tile([1, 512], F32, tag="s1", name="s1", bufs=2)
                nc.tensor.matmul(s1p, ones_col, xm[:, sl], start=True, stop=True)
                s2p = ps.tile([1, 512], F32, tag="s2", name="s2", bufs=2)
                nc.tensor.matmul(s2p, ones_col, xsq, start=True, stop=True)
                # row-resident LN stats: mu, var, rs = 1/sqrt(var+eps), ms = mu*rs
                mu = wrk.tile([1, 512], F32, tag="mu", name="mu")
                nc.vector.tensor_scalar(mu, s1p, 1.0 / C, None, op0=OP.mult)
                m2 = wrk.tile([1, 512], F32, tag="m2", name="m2")
                nc.vector.tensor_scalar(m2, s2p, 1.0 / C, None, op0=OP.mult)
                musq = wrk.tile([1, 512], F32, tag="musq", name="musq")
                nc.gpsimd.tensor_tensor(out=musq, in0=mu, in1=mu, op=OP.mult)
                nc.gpsimd.tensor_tensor(out=m2, in0=m2, in1=musq, op=OP.subtract)
                sd = wrk.tile([1, 512], F32, tag="sd", name="sd")
                nc.scalar.activation(sd, m2, AF.Sqrt, bias=eps_row[:, 0:1])
                rs = wrk.tile([1, 512], F32, tag="rs", name="rs")
                nc.vector.reciprocal(rs, sd)
                rs_bf = wrk.tile([1, 512], BF16, tag="rsb", name="rsb")
                nc.vector.tensor_copy(rs_bf, rs)
                ms_bf = wrk.tile([1, 512], BF16, tag="msb", name="msb")
                nc.gpsimd.tensor_tensor(out=ms_bf, in0=mu, in1=rs, op=OP.mult)
                prep = ps.tile([128, 512], F32, tag="prep", name="prep", bufs=2)
                nc.tensor.matmul(prep, ones_p1, rs_bf, start=True, stop=True)
                xms = wrk.tile([128, 512], BF16, tag="xms", name="xms")
                nc.vector.tensor_tensor(out=xms, in0=xm[:, sl], in1=prep, op=OP.mult)
                po2 = ps.tile([128, 512], F32, tag="po2", name="po2", bufs=2)
                nc.tensor.matmul(po2, wpg_sb, xms, start=True, stop=False)
                nc.tensor.matmul(po2, w1_sb, ms_bf, start=False, stop=False, skip_group_check=True)
                nc.tensor.matmul(po2, wbp_sb, ones_row[:, sl], start=False, stop=True, skip_group_check=True)
                nc.scalar.copy(out_f[:, sl], po2)
                nc.sync.dma_start(out=fout[:, sl], in_=out_f[:, sl])

    orig = nc.to_json_bytes
    nc.to_json_bytes = lambda: _split_waits(orig())
    return nc


_CACHE = {}


def _get_nc(a_vals):
    key = tuple(np.round(np.asarray(a_vals, np.float64), 9))
    if key not in _CACHE:
        _CACHE[key] = build_nc(key)
    return _CACHE[key]


def _layernorm_c(x, gamma, beta):
    """x: (B, C, L) fp32, normalize over C."""
    x = x.astype(np.float32)
    mu = x.mean(axis=1, keepdims=True, dtype=np.float32)
    xc = x - mu
    var = np.mean(xc * xc, axis=1, keepdims=True, dtype=np.float32)
    xn = xc / np.sqrt(var + np.float32(EPS))
    return xn * gamma.astype(np.float32)[None, :, None] + beta.astype(np.float32)[None, :, None]


def kernel(**inputs):
    inp = {k: np.asarray(v) for k, v in inputs.items()}
    x = inp["x"].astype(np.float32)
    gamma, beta = inp["gamma"], inp["beta"]
    s1 = float(np.asarray(inp["s1"]).reshape(-1)[0])
    s2 = float(np.asarray(inp["s2"]).reshape(-1)[0])

    xb = x.reshape(B, C, L)
    perm = np.array([(j % GROUP) * (C // GROUP) + j // GROUP for j in range(C)])
    x1 = _layernorm_c(xb, gamma, beta)              # (B, C, L)
    x2 = _layernorm_c(xb[:, perm, :], gamma, beta)  # (B, C, L)

    a_vals = -np.exp(inp["A_log"][0].astype(np.float64))  # (16,)
    nc = _get_nc(a_vals)

    bf = lambda a: np.ascontiguousarray(np.asarray(a, np.float32).astype(BF))
    f32 = lambda a: np.ascontiguousarray(a, np.float32)
    W_in = inp["W_in"].astype(np.float64)           # (512, 128)
    conv_w = inp["conv_w"][:, 0, :].astype(np.float64)  # (256, 4)
    # conv fused into in_proj: wk[i,k] = (diag(conv_w[i-block,k]) @ W_in_block_i).T
    wk = np.empty((2, D_CONV, C, 128), np.float64)
    for i in range(2):
        blk = W_in[i * 128:(i + 1) * 128, :]        # (128, C)
        for k in range(D_CONV):
            wk[i, k] = (conv_w[i * 128:(i + 1) * 128, k][:, None] * blk).T
    wz = np.stack([W_in[DI + i * 128: DI + (i + 1) * 128, :].T for i in range(2)])
    W_x = inp["W_x"].astype(np.float64)             # (40, 256)
    wx = np.stack([W_x[:, i * 128:(i + 1) * 128].T for i in range(2)])
    W_dt = inp["W_dt"].astype(np.float64)           # (256, 8)
    wdt = np.stack([W_dt[i * 128:(i + 1) * 128, :].T for i in range(2)])
    W_out = inp["W_out"].astype(np.float64)         # (128, 256)
    wout = np.stack([W_out[:, i * 128:(i + 1) * 128].T for i in range(2)])
    D_skip = inp["D_skip"].astype(np.float64)

    NXD = DT_RANK + 2 * N_STATE
    wdt_pad = np.zeros((2, C, 128), np.float64)
    wdt_pad[:, :DT_RANK, :] = wdt
    wx_cols = np.concatenate([wx[0], wx[1]], axis=1)
    # order: wk(8) wz(2) wout(2) ident identd(2) idents wdt(2) | wx cols
    cols = [wk[i, k] for i in range(2) for k in range(D_CONV)]
    cols += [wz[0], wz[1], wout[0], wout[1], np.eye(128),
             np.diag(D_skip[:128]), np.diag(D_skip[128:]), np.zeros((C, 128))]
    cols += [wdt_pad[0], wdt_pad[1]]
    wpack = np.concatenate(cols + [np.zeros((C, 0))], axis=1)
    wpack = np.concatenate([wpack, wx_cols], axis=1)
    bpack = np.stack([inp["conv_b"][:128], inp["conv_b"][128:],
                      inp["b_dt"][:128], inp["b_dt"][128:]], axis=1).astype(np.float32)
    weights = dict(wpack=bf(wpack), bpack=f32(bpack))
    pad = np.zeros((B, C, D_CONV - 1), np.float32)
    in_maps = []
    for br, xbr, s in ((0, x1, s1), (1, x2, s2)):
        wp = np.array(weights["wpack"])
        wp[:, 15 * 128:16 * 128] = (s * np.eye(128)).astype(BF)
        for b in range(B):
            m = dict(weights)
            m["wpack"] = wp
            m["xin"] = bf(np.concatenate([pad[b], xbr[b]], axis=1))
            in_maps.append(m)

    res = run_bass_kernel_spmd(nc, in_maps, core_ids=list(range(8)))
    partials = [r["pout"] for r in res.results]  # (C, L) bf16 each

    # phase 2 on device: 8 cores = 4 batches x 2 half-sequences
    if "nc2" not in _CACHE:
        _CACHE["nc2"] = build_nc2()
    nc2 = _CACHE["nc2"]
    W_p = inp["W_p"].astype(np.float64)
    wpg = W_p * gamma.astype(np.float64)[None, :]                        # (out, C)
    w1 = -wpg.sum(axis=1)                                                # (out,)
    wbp = inp["b_p"].astype(np.float64) + W_p @ beta.astype(np.float64)
    w2 = dict(
        wpg_T=bf(wpg.T),
        w1r=bf(w1.reshape(1, C)),
        wbp=bf(wbp.reshape(1, C)),
    )
    in_maps2 = []
    for b in range(B):
        for h in range(2):
            m = dict(w2)
            sl = slice(h * LH, (h + 1) * LH)
            m["pa"] = np.ascontiguousarray(partials[b][:, sl])
            m["pb"] = np.ascontiguousarray(partials[4 + b][:, sl])
            in_maps2.append(m)
    res2 = run_bass_kernel_spmd(nc2, in_maps2, core_ids=list(range(8)))
    out = np.empty((B, C, L), np.float32)
    for b in range(B):
        for h in range(2):
            out[b][:, h * LH:(h + 1) * LH] = res2.results[b * 2 + h]["fout"]
    return out.reshape(B, -1, W_, H_)
